# revision 2
# baseline (speedup 1.0000x reference)
"""GAT (2-layer, 4-head) kernel for 8 Trainium2 NeuronCores.

Strategy (graph/data parallel, dst-sharded):
  - Nodes sharded 8 ways by destination; edges owned by dst shard.
  - Encoder runs per-shard; per-node gather tables (features + folded
    attention-source logits, bf16, 256B rows) are AllGathered.
  - Per 128-dst window: dma_gather of source rows, one-hot segment
    matmuls on PE accumulate softmax numerator/denominator in PSUM.
  - GAT layer 1 aggregates RAW encoder features (projection is applied
    after aggregation -- attention weights are per-head scalars so
    sum(a*(W h)) == W sum(a*h)), making gather rows 64 floats.
  - Layer 2 projects first (4x16 heads), gathers projected rows.
  - Softmax uses the one-pass form num/(sum(exp(z)) + eps); z range is
    tiny (checked numerically) so no max-subtraction is needed.
"""

import math
import os

import numpy as np

H = 4  # attention heads
P = 128
EPS = 1e-16
NEG_SLOPE = 0.2
TSPLIT = 32768  # int16 gather-index ceiling (split tables into A/B halves)
ENC_CHUNK = 512
SUP = 4  # windows per gather super-batch

LAST_RESULTS = None  # BassKernelResults of the most recent run (for test.py)
DBG_SKIP = set()  # debug: stage names to replace with memset/no-op


def _f32(x):
    return np.ascontiguousarray(x, dtype=np.float32)


def _prep_host(inputs):
    """All host-side preprocessing: sharding, edge partitioning, padding."""
    import ml_dtypes

    x = _f32(inputs["x"])
    ei = np.asarray(inputs["edge_index"])
    N = x.shape[0]
    NC = 8
    assert N % NC == 0
    shard = N // NC
    w_pc = math.ceil(shard / P)  # windows per core
    shard_pad = w_pc * P
    rows_total = NC * shard_pad

    loop = np.arange(N, dtype=np.int64)
    src = np.concatenate([ei[0].astype(np.int64), loop])
    dst = np.concatenate([ei[1].astype(np.int64), loop])

    # table row id for a global node: shards are padded to shard_pad rows
    trow = (src // shard) * shard_pad + (src % shard)

    has_b = rows_total > TSPLIT

    per_core = []
    max_a = 0
    max_b = 0
    for r in range(NC):
        sel = (dst >= r * shard) & (dst < (r + 1) * shard)
        tr, dl = trow[sel], dst[sel] - r * shard
        order = np.argsort(dl, kind="stable")
        tr, dl = tr[order], dl[order]
        wins = []
        for w in range(w_pc):
            m = (dl >= w * P) & (dl < (w + 1) * P)
            trw, dlw = tr[m], dl[m] - w * P
            a = trw < TSPLIT
            wins.append((trw[a], dlw[a], trw[~a] - TSPLIT, dlw[~a]))
            max_a = max(max_a, trw[a].size)
            max_b = max(max_b, trw.size - trw[a].size)
        per_core.append(wins)

    t_a = max(P, math.ceil(max_a / P) * P)
    t_b = max(P, math.ceil(max_b / P) * P) if has_b else 0
    sl_a, sl_b = t_a // P, t_b // P
    s_w = sl_a + sl_b
    t_w = t_a + t_b

    def wrap16(v, cap):
        out = np.zeros((16, cap // 16), np.int16)
        out[np.arange(v.size) % 16, np.arange(v.size) // 16] = v
        return out

    def wrap128(v, cap):
        out = np.full((P, cap // P), -1.0, np.float32)
        out[np.arange(v.size) % P, np.arange(v.size) // P] = v
        return out

    cores = []
    for r in range(NC):
        idx_a = np.zeros((P, w_pc * (t_a // 16)), np.int16)
        idx_b = np.zeros((P, w_pc * (t_b // 16)), np.int16) if has_b else None
        dwrap = np.full((P, w_pc * s_w), -1.0, np.float32)
        dflat = np.full((w_pc, t_w), -1.0, np.float32)
        for w, (ta, da, tb, db) in enumerate(per_core[r]):
            wa = wrap16(ta, t_a)
            idx_a[:, w * (t_a // 16):(w + 1) * (t_a // 16)] = np.tile(wa, (8, 1))
            dwrap[:, w * s_w: w * s_w + sl_a] = wrap128(da, t_a)
            dflat[w, :da.size] = da
            if has_b:
                wb = wrap16(tb, t_b)
                idx_b[:, w * (t_b // 16):(w + 1) * (t_b // 16)] = np.tile(wb, (8, 1))
                dwrap[:, w * s_w + sl_a:(w + 1) * s_w] = wrap128(db, t_b)
            dflat[w, t_a:t_a + db.size] = db

        xt = np.zeros((7, shard_pad), np.float32)
        xt[:, :shard] = x[r * shard:(r + 1) * shard].T
        core = {
            "xT": xt,
            "idxA": idx_a,
            "dstwrap": dwrap,
            "dstflat": np.repeat(dflat.astype(ml_dtypes.bfloat16)[:, None, :], P, axis=1),
        }
        if has_b:
            core["idxB"] = idx_b
        cores.append(core)

    # ---- weights / constants (replicated) ----
    g1w = _f32(inputs["g1_w"])            # [64, 256]
    a1s, a1d = _f32(inputs["g1_as"]), _f32(inputs["g1_ad"])   # [H,64]
    g2w = _f32(inputs["g2_w"])            # [256, 64]
    a2s, a2d = _f32(inputs["g2_as"]), _f32(inputs["g2_ad"])   # [H,16]
    c1 = g1w.shape[1] // H                # 64
    c2 = g2w.shape[1] // H                # 16

    atil_s = np.stack([g1w[:, h * c1:(h + 1) * c1] @ a1s[h] for h in range(H)], 1)
    atil_d = np.stack([g1w[:, h * c1:(h + 1) * c1] @ a1d[h] for h in range(H)], 1)
    asd1 = np.concatenate([atil_s, atil_d], 1)  # [64, 8]

    m2s = np.zeros((H * c2, H), np.float32)
    m2d = np.zeros((H * c2, H), np.float32)
    for h in range(H):
        m2s[h * c2:(h + 1) * c2, h] = a2s[h]
        m2d[h * c2:(h + 1) * c2, h] = a2d[h]
    m2sd = np.concatenate([m2s, m2d], 1)  # [64, 8]

    b1 = _f32(inputs["g1_b"])             # [256]
    g2wsb = np.concatenate([g2w[:P], g2w[P:]], 1)  # [128, 128]
    negc2 = -(g2w.sum(0))[:, None]        # [64, 1]

    consts = {
        "enc_w1": _f32(inputs["enc_w1"]),
        "enc_b1c": _f32(inputs["enc_b1"])[:, None],
        "enc_w2": _f32(inputs["enc_w2"]),
        "enc_b2c": _f32(inputs["enc_b2"])[:, None],
        "asd1": asd1,
        "g1wb": g1w,
        "b1cols": b1.reshape(2, P).T.copy(),  # [128, 2]
        "g2wsb": g2wsb,
        "negc2": negc2,
        "m2sd": m2sd,
        "g2bc": _f32(inputs["g2_b"])[:, None],
        "dw1": _f32(inputs["dec_w1"]),
        "db1c": _f32(inputs["dec_b1"])[:, None],
        "dw2": _f32(inputs["dec_w2"]),
        "db2c": _f32(inputs["dec_b2"])[:, None],
        "iota_row": np.broadcast_to(np.arange(P, dtype=np.float32), (P, P)).copy(),
        "iota_col": np.arange(P, dtype=np.float32)[:, None].astype(ml_dtypes.bfloat16),
        "identity": np.eye(P, dtype=np.float32),
    }

    geom = dict(N=N, NC=NC, shard=shard, w_pc=w_pc, shard_pad=shard_pad,
                rows_total=rows_total, has_b=has_b, t_a=t_a, t_b=t_b,
                sl_a=sl_a, sl_b=sl_b, s_w=s_w, t_w=t_w, c1=c1, c2=c2)
    return geom, cores, consts


def _build(geom):
    """Emit the Bass/Tile program (identical for all cores)."""
    import concourse.bass as bass
    import concourse.mybir as mybir
    import concourse.tile as tile
    from concourse import bacc

    dt = mybir.dt
    AF = mybir.ActivationFunctionType
    OP = mybir.AluOpType

    NC, w_pc, shard_pad = geom["NC"], geom["w_pc"], geom["shard_pad"]
    rows_total, has_b = geom["rows_total"], geom["has_b"]
    t_a, t_b, sl_a, sl_b = geom["t_a"], geom["t_b"], geom["sl_a"], geom["sl_b"]
    s_w, t_w, c1, c2 = geom["s_w"], geom["t_w"], geom["c1"], geom["c2"]
    rg = [list(range(NC))]

    nc = bacc.Bacc("TRN2", target_bir_lowering=False, debug=False,
                   enable_asserts=False, num_devices=NC)

    # ---- I/O ----
    xT = nc.dram_tensor("xT", [7, shard_pad], dt.float32, kind="ExternalInput").ap()
    idxA = nc.dram_tensor("idxA", [P, w_pc * (t_a // 16)], dt.int16,
                          kind="ExternalInput").ap()
    idxB = (nc.dram_tensor("idxB", [P, w_pc * (t_b // 16)], dt.int16,
                           kind="ExternalInput").ap() if has_b else None)
    dstwrap = nc.dram_tensor("dstwrap", [P, w_pc * s_w], dt.float32,
                             kind="ExternalInput").ap()
    dstflat = nc.dram_tensor("dstflat", [w_pc, P, t_w], dt.bfloat16,
                             kind="ExternalInput").ap()
    cns = {}
    for name, shape in [
        ("enc_w1", [7, 64]), ("enc_b1c", [64, 1]), ("enc_w2", [64, 64]),
        ("enc_b2c", [64, 1]), ("asd1", [64, 2 * H]), ("g1wb", [64, H * c1]),
        ("b1cols", [P, 2]), ("g2wsb", [P, P]), ("negc2", [64, 1]),
        ("m2sd", [64, 2 * H]), ("g2bc", [64, 1]), ("dw1", [64, 64]),
        ("db1c", [64, 1]), ("dw2", [64, H]), ("db2c", [H, 1]),
        ("iota_row", [P, P]), ("identity", [P, P]),
    ]:
        cns[name] = nc.dram_tensor(name, shape, dt.float32,
                                   kind="ExternalInput").ap()
    cns["iota_col"] = nc.dram_tensor("iota_col", [P, 1], dt.bfloat16,
                                     kind="ExternalInput").ap()
    out = nc.dram_tensor("out", [shard_pad, H], dt.float32,
                         kind="ExternalOutput").ap()

    supers = [list(range(s, min(s + SUP, w_pc))) for s in range(0, w_pc, SUP)]
    enc_chunks = []
    c0 = 0
    while c0 < shard_pad:
        cw = min(ENC_CHUNK, shard_pad - c0)
        enc_chunks.append((c0, cw))
        c0 += cw

    with tile.TileContext(nc) as tc:
        with tc.tile_pool(name="dram", bufs=1, space="DRAM") as dram:
            t1_shard = dram.tile([shard_pad, P], dt.bfloat16)
            t1_full = dram.tile([rows_total, P], dt.bfloat16, addr_space="Shared")
            t2_shard = dram.tile([shard_pad, P], dt.bfloat16)
            t2_full = dram.tile([rows_total, P], dt.bfloat16, addr_space="Shared")

            with tc.tile_pool(name="cpool", bufs=1) as cpool:
                csb = {}
                for name, ap in cns.items():
                    t = cpool.tile(ap.shape, ap.dtype, name=f"c_{name}", tag=f"c_{name}")
                    nc.sync.dma_start(t[:], ap)
                    csb[name] = t
                idxA_sb = cpool.tile(idxA.shape, dt.int16, name="idxA_sb", tag="idxA_sb")
                nc.sync.dma_start(idxA_sb[:], idxA)
                if has_b:
                    idxB_sb = cpool.tile(idxB.shape, dt.int16, name="idxB_sb", tag="idxB_sb")
                    nc.sync.dma_start(idxB_sb[:], idxB)
                dwrap_sb = cpool.tile(dstwrap.shape, dt.float32, name="dwrap_sb", tag="dwrap_sb")
                nc.sync.dma_start(dwrap_sb[:], dstwrap)
                adst1_sb = cpool.tile([P, H * w_pc], dt.bfloat16, name="adst1_sb", tag="adst1_sb")
                adst2_sb = cpool.tile([P, H * w_pc], dt.bfloat16, name="adst2_sb", tag="adst2_sb")

                ident = csb["identity"]

                # ================= encoder =================
                with tc.tile_pool(name="encs", bufs=2) as encs, \
                     tc.tile_pool(name="encp", bufs=2, space="PSUM") as encp, \
                     tc.tile_pool(name="encp1", bufs=1, space="PSUM") as encp1:
                    xT_sb = encs.tile([7, shard_pad], dt.float32, tag="xT_sb", bufs=1)
                    nc.sync.dma_start(xT_sb[:], xT)
                    for (c0, cw) in enc_chunks:
                        nb = cw // P
                        ps1 = encp.tile([64, ENC_CHUNK], dt.float32, tag="ps1")
                        nc.tensor.matmul(ps1[:, :cw], csb["enc_w1"][:],
                                         xT_sb[:, c0:c0 + cw], start=True, stop=True)
                        h1a = encs.tile([64, ENC_CHUNK], dt.float32, tag="h1a")
                        nc.scalar.activation(h1a[:, :cw], ps1[:, :cw], AF.Relu,
                                             bias=csb["enc_b1c"][:])
                        ps2 = encp.tile([64, ENC_CHUNK], dt.float32, tag="ps2")
                        nc.tensor.matmul(ps2[:, :cw], csb["enc_w2"][:],
                                         h1a[:, :cw], start=True, stop=True)
                        h1T = encs.tile([64, ENC_CHUNK], dt.float32, tag="h1T")
                        nc.scalar.activation(h1T[:, :cw], ps2[:, :cw], AF.Identity,
                                             bias=csb["enc_b2c"][:])
                        ps8 = encp1.tile([2 * H, ENC_CHUNK], dt.float32, tag="ps8")
                        nc.tensor.matmul(ps8[:, :cw], csb["asd1"][:],
                                         h1T[:, :cw], start=True, stop=True)
                        sb8 = encs.tile([2 * H, ENC_CHUNK], dt.float32, tag="sb8")
                        nc.vector.tensor_copy(sb8[:, :cw], ps8[:, :cw])

                        pst = encp.tile([P, 64 * (ENC_CHUNK // P)], dt.float32, tag="pst")
                        pst8 = encp1.tile([P, 2 * H * (ENC_CHUNK // P)], dt.float32, tag="pst8")
                        for b in range(nb):
                            nc.tensor.transpose(pst[:, 64 * b:64 * (b + 1)],
                                                h1T[:, b * P:(b + 1) * P],
                                                ident[:64, :64])
                            nc.tensor.transpose(pst8[:, 2 * H * b:2 * H * (b + 1)],
                                                sb8[:, b * P:(b + 1) * P],
                                                ident[:2 * H, :2 * H])
                        pk = encs.tile([P, (ENC_CHUNK // P) * P], dt.bfloat16, tag="pk")
                        nc.gpsimd.memset(pk[:, :nb * P], 0.0)
                        nc.vector.tensor_copy(
                            pk[:, :nb * P].rearrange("p (b c) -> p b c", b=nb)[:, :, 0:64],
                            pst[:, :64 * nb].rearrange("p (b c) -> p b c", b=nb))
                        nc.vector.tensor_copy(
                            pk[:, :nb * P].rearrange("p (b c) -> p b c", b=nb)[:, :, 64:64 + H],
                            pst8[:, :2 * H * nb].rearrange("p (b c) -> p b c", b=nb)[:, :, 0:H])
                        nc.vector.tensor_copy(
                            adst1_sb[:, H * (c0 // P): H * (c0 // P + nb)],
                            pst8[:, :2 * H * nb].rearrange("p (b c) -> p b c", b=nb)[:, :, H:2 * H])
                        nc.sync.dma_start(
                            t1_shard[c0:c0 + cw, :].rearrange("(b p) c -> p b c", p=P),
                            pk[:, :nb * P].rearrange("p (b c) -> p b c", b=nb))

                nc.gpsimd.collective_compute(
                    "AllGather", mybir.AluOpType.bypass, replica_groups=rg,
                    ins=[t1_shard[:].opt()], outs=[t1_full[:].opt()])

                # ================= GAT layers =================
                def gat_layer(layer):
                    ch = c1 if layer == 1 else c2          # per-head width
                    fw = H * ch + H                        # rhs width per slot
                    tfull = t1_full if layer == 1 else t2_full
                    adst_sb = adst1_sb if layer == 1 else adst2_sb

                    with tc.tile_pool(name=f"gs{layer}", bufs=2) as gs, \
                         tc.tile_pool(name=f"gs1{layer}", bufs=1) as gs1, \
                         tc.tile_pool(name=f"gpA{layer}", bufs=2, space="PSUM") as gpA, \
                         tc.tile_pool(name=f"gpB{layer}", bufs=1, space="PSUM") as gpB:
                        for sup in supers:
                            nw = len(sup)
                            w0 = sup[0]
                            hgA = gs.tile([P, SUP * sl_a * P], dt.bfloat16, tag="hgA")
                            if "gath" in DBG_SKIP:
                                nc.vector.memset(hgA[:, :nw * sl_a * P], 0.25)
                            else:
                                nc.gpsimd.dma_gather(
                                    out_ap=hgA[:, :nw * sl_a * P].rearrange(
                                        "p (s c) -> p s c", c=P),
                                    in_ap=tfull[0:TSPLIT if has_b else rows_total, :],
                                    idxs_ap=idxA_sb[:, w0 * (t_a // 16):(w0 + nw) * (t_a // 16)],
                                    num_idxs=nw * t_a, num_idxs_reg=nw * t_a,
                                    elem_size=P, single_packet=False)
                            if has_b:
                                hgB = gs.tile([P, SUP * sl_b * P], dt.bfloat16, tag="hgB")
                                if "gath" in DBG_SKIP:
                                    nc.vector.memset(hgB[:, :nw * sl_b * P], 0.25)
                                else:
                                    nc.gpsimd.dma_gather(
                                        out_ap=hgB[:, :nw * sl_b * P].rearrange(
                                            "p (s c) -> p s c", c=P),
                                        in_ap=tfull[TSPLIT:rows_total, :],
                                        idxs_ap=idxB_sb[:, w0 * (t_b // 16):(w0 + nw) * (t_b // 16)],
                                        num_idxs=nw * t_b, num_idxs_reg=nw * t_b,
                                        elem_size=P, single_packet=False)
                            dfl = gs.tile([P, SUP * t_w], dt.bfloat16, tag="dfl", bufs=1)
                            nc.sync.dma_start(
                                dfl[:, :nw * t_w].rearrange("p (a b) -> p a b", a=nw),
                                dstflat[w0:w0 + nw, :, :].transpose([1, 0, 2]))
                            if layer == 1:
                                pk2 = gs1.tile([P, SUP * P], dt.bfloat16, tag="pk2")
                                nc.gpsimd.memset(pk2[:, :nw * P], 0.0)
                            else:
                                opk = gs1.tile([P, SUP * H], dt.float32, tag="opk")

                            for wl, w in enumerate(sup):
                                # ---- S matrices ----
                                s_all = gs.tile([P, s_w * P], dt.bfloat16, tag="s_all")
                                nc.vector.tensor_tensor(
                                    out=s_all[:].rearrange("p (s c) -> p s c", c=P),
                                    in0=dwrap_sb[:, w * s_w:(w + 1) * s_w]
                                        .unsqueeze(2).to_broadcast([P, s_w, P]),
                                    in1=csb["iota_row"][:].unsqueeze(1)
                                        .to_broadcast([P, s_w, P]),
                                    op=OP.is_equal)
                                s_t = gs.tile([P, t_w], dt.bfloat16, tag="s_t")
                                nc.vector.tensor_tensor(
                                    out=s_t[:],
                                    in0=dfl[:, wl * t_w:(wl + 1) * t_w],
                                    in1=csb["iota_col"][:].to_broadcast([P, t_w]),
                                    op=OP.is_equal)
                                # ---- z = asrc + adst ----
                                zps = gpA.tile([P, H * s_w], dt.float32, tag="zps")
                                if "zmm" in DBG_SKIP:
                                    nc.vector.memset(zps[:], 0.0)
                                else:
                                    for j in range(s_w):
                                        nc.tensor.matmul(
                                            zps[:, H * j:H * (j + 1)],
                                            s_t[:, P * j:P * (j + 1)],
                                            adst_sb[:, H * w:H * (w + 1)],
                                            start=True, stop=True)
                                zsb = gs.tile([P, H * s_w], dt.float32, tag="zsb")
                                nc.vector.tensor_add(
                                    zsb[:, :H * sl_a].rearrange("p (s h) -> p s h", h=H),
                                    zps[:, :H * sl_a].rearrange("p (s h) -> p s h", h=H),
                                    hgA[:, wl * sl_a * P:(wl + 1) * sl_a * P]
                                        .rearrange("p (s c) -> p s c", c=P)[:, :, 64:64 + H])
                                if has_b:
                                    nc.vector.tensor_add(
                                        zsb[:, H * sl_a:].rearrange("p (s h) -> p s h", h=H),
                                        zps[:, H * sl_a:].rearrange("p (s h) -> p s h", h=H),
                                        hgB[:, wl * sl_b * P:(wl + 1) * sl_b * P]
                                            .rearrange("p (s c) -> p s c", c=P)[:, :, 64:64 + H])
                                # ---- w = exp(leaky_relu(z)) ----
                                lr = gs.tile([P, H * s_w], dt.float32, tag="lr")
                                nc.vector.scalar_tensor_tensor(
                                    out=lr[:], in0=zsb[:], scalar=NEG_SLOPE,
                                    in1=zsb[:], op0=OP.mult, op1=OP.max)
                                wsb = gs.tile([P, H * s_w], dt.float32, tag="wsb")
                                nc.scalar.activation(wsb[:], lr[:], AF.Exp)
                                # ---- scaled messages ----
                                sh = gs.tile([P, s_w * fw], dt.bfloat16, tag="sh")
                                shv = sh[:].rearrange("p (s f) -> p s f", f=fw)
                                if layer == 1:
                                    in0a = (hgA[:, wl * sl_a * P:(wl + 1) * sl_a * P]
                                            .rearrange("p (s c) -> p s c", c=P)[:, :, 0:ch]
                                            .unsqueeze(2).to_broadcast([P, sl_a, H, ch]))
                                    in0b = (hgB[:, wl * sl_b * P:(wl + 1) * sl_b * P]
                                            .rearrange("p (s c) -> p s c", c=P)[:, :, 0:ch]
                                            .unsqueeze(2).to_broadcast([P, sl_b, H, ch])
                                            if has_b else None)
                                else:
                                    in0a = (hgA[:, wl * sl_a * P:(wl + 1) * sl_a * P]
                                            .rearrange("p (s c) -> p s c", c=P)
                                            [:, :, 0:H * ch]
                                            .rearrange("p s (h c) -> p s h c", h=H))
                                    in0b = (hgB[:, wl * sl_b * P:(wl + 1) * sl_b * P]
                                            .rearrange("p (s c) -> p s c", c=P)
                                            [:, :, 0:H * ch]
                                            .rearrange("p s (h c) -> p s h c", h=H)
                                            if has_b else None)
                                wv = wsb[:].rearrange("p (s h) -> p s h", h=H)
                                nc.vector.tensor_mul(
                                    shv[:, 0:sl_a, 0:H * ch].rearrange(
                                        "p s (h c) -> p s h c", h=H),
                                    in0a,
                                    wv[:, 0:sl_a].unsqueeze(3).to_broadcast(
                                        [P, sl_a, H, ch]))
                                if has_b:
                                    nc.vector.tensor_mul(
                                        shv[:, sl_a:s_w, 0:H * ch].rearrange(
                                            "p s (h c) -> p s h c", h=H),
                                        in0b,
                                        wv[:, sl_a:s_w].unsqueeze(3).to_broadcast(
                                            [P, sl_b, H, ch]))
                                nc.vector.tensor_copy(shv[:, :, H * ch:fw], wv)
                                # ---- aggregation ----
                                agg = gpA.tile([P, fw], dt.float32, tag="agg")
                                if "aggmm" in DBG_SKIP:
                                    nc.vector.memset(agg[:], 1.0)
                                else:
                                    for j in range(s_w):
                                        nc.tensor.matmul(
                                            agg[:], s_all[:, P * j:P * (j + 1)],
                                            sh[:, fw * j:fw * (j + 1)],
                                            start=(j == 0), stop=(j == s_w - 1))
                                den = gs.tile([P, H], dt.float32, tag="den")
                                nc.vector.tensor_scalar_add(den[:], agg[:, H * ch:H * ch + H], EPS)
                                rec = gs.tile([P, H], dt.float32, tag="rec")
                                nc.vector.reciprocal(rec[:], den[:])
                                asb = gs.tile([P, H * ch], dt.float32, tag="asb")
                                nc.vector.tensor_mul(
                                    asb[:].rearrange("p (h c) -> p h c", h=H),
                                    agg[:, 0:H * ch].rearrange("p (h c) -> p h c", h=H),
                                    rec[:].unsqueeze(2).to_broadcast([P, H, ch]))

                                if layer == 1:
                                    _post1(nc, tc, csb, gs, gpB, asb, pk2, adst2_sb,
                                           wl, w, ident, dt, AF, OP)
                                else:
                                    _post2(nc, csb, gs, gpB, asb, opk, wl, ident,
                                           dt, AF)
                            if layer == 1:
                                nc.sync.dma_start(
                                    t2_shard[P * w0:P * (w0 + nw), :].rearrange(
                                        "(b p) c -> p b c", p=P),
                                    pk2[:, :nw * P].rearrange("p (b c) -> p b c", b=nw))
                            else:
                                nc.sync.dma_start(
                                    out[P * w0:P * (w0 + nw), :].rearrange(
                                        "(b p) c -> p b c", p=P),
                                    opk[:, :nw * H].rearrange("p (b c) -> p b c", b=nw))

                gat_layer(1)
                nc.gpsimd.collective_compute(
                    "AllGather", mybir.AluOpType.bypass, replica_groups=rg,
                    ins=[t2_shard[:].opt()], outs=[t2_full[:].opt()])
                gat_layer(2)

    nc.compile()
    return nc


def _post1(nc, tc, csb, gs, gpB, asb, pk2, adst2_sb, wl, w, ident, dt, AF, OP):
    """Layer-1 per-window tail: project, ELU(+1), layer-2 projection,
    attention logits, pack bf16 table rows."""
    c1 = 64
    t4 = gpB.tile([64, H * P], dt.float32, tag="pscr")
    for h in range(H):
        nc.tensor.transpose(t4[:, P * h:P * (h + 1)],
                            asb[:, c1 * h:c1 * (h + 1)], ident[:, :])
    aggT = gs.tile([64, H * P], dt.float32, tag="aggT")
    nc.vector.tensor_copy(aggT[:], t4[:])
    o1a = gpB.tile([P, P], dt.float32, tag="o1a")
    o1b = gpB.tile([P, P], dt.float32, tag="o1b")
    for h in range(H):
        tgt = o1a if h < 2 else o1b
        nc.tensor.matmul(tgt[64 * (h % 2):64 * (h % 2) + 64, :],
                         csb["g1wb"][:, 64 * h:64 * (h + 1)],
                         aggT[:, P * h:P * (h + 1)], start=True, stop=True)
    h2p = []
    for t, ps in enumerate((o1a, o1b)):
        ex = gs.tile([P, P], dt.float32, tag=f"ex{t}")
        nc.scalar.activation(ex[:], ps[:], AF.Exp, bias=csb["b1cols"][:, t:t + 1])
        rl = gs.tile([P, P], dt.float32, tag=f"rl{t}")
        nc.scalar.activation(rl[:], ps[:], AF.Relu, bias=csb["b1cols"][:, t:t + 1])
        hp = gs.tile([P, P], dt.float32, tag=f"hp{t}")
        nc.vector.scalar_tensor_tensor(out=hp[:], in0=ex[:], scalar=1.0,
                                       in1=rl[:], op0=OP.min, op1=OP.add)
        h2p.append(hp)
    h2pT = gpB.tile([64, P], dt.float32, tag="pscr")
    nc.tensor.matmul(h2pT[:], csb["g2wsb"][:, 0:64], h2p[0][:], start=True, stop=False)
    nc.tensor.matmul(h2pT[:], csb["g2wsb"][:, 64:128], h2p[1][:], start=False, stop=True)
    h2pTs = gs.tile([64, P], dt.float32, tag="h2pTs")
    nc.scalar.activation(h2pTs[:], h2pT[:], AF.Identity, bias=csb["negc2"][:])
    ps8 = gpB.tile([2 * H, P], dt.float32, tag="pscr")
    nc.tensor.matmul(ps8[:], csb["m2sd"][:], h2pTs[:], start=True, stop=True)
    sb8 = gs.tile([2 * H, P], dt.float32, tag="sb8w")
    nc.vector.tensor_copy(sb8[:], ps8[:])
    tp = gpB.tile([P, 64], dt.float32, tag="pscr")
    nc.tensor.transpose(tp[:], h2pTs[:], ident[:64, :64])
    tp8 = gpB.tile([P, 2 * H], dt.float32, tag="pscr2")
    nc.tensor.transpose(tp8[:], sb8[:], ident[:2 * H, :2 * H])
    nc.vector.tensor_copy(pk2[:, P * wl:P * wl + 64], tp[:])
    nc.vector.tensor_copy(pk2[:, P * wl + 64:P * wl + 64 + H], tp8[:, 0:H])
    nc.vector.tensor_copy(adst2_sb[:, H * w:H * (w + 1)], tp8[:, H:2 * H])


def _post2(nc, csb, gs, gpB, asb, opk, wl, ident, dt, AF):
    """Layer-2 per-window tail: bias, decoder MLP, transpose out rows."""
    o2T = gpB.tile([64, P], dt.float32, tag="pscr")
    nc.tensor.transpose(o2T[:], asb[:], ident[:, :])
    o2Ts = gs.tile([64, P], dt.float32, tag="o2Ts")
    nc.scalar.activation(o2Ts[:], o2T[:], AF.Identity, bias=csb["g2bc"][:])
    d1 = gpB.tile([64, P], dt.float32, tag="pscr2")
    nc.tensor.matmul(d1[:], csb["dw1"][:], o2Ts[:], start=True, stop=True)
    d1s = gs.tile([64, P], dt.float32, tag="d1s")
    nc.scalar.activation(d1s[:], d1[:], AF.Relu, bias=csb["db1c"][:])
    d2 = gpB.tile([H, P], dt.float32, tag="pscr")
    nc.tensor.matmul(d2[:], csb["dw2"][:], d1s[:], start=True, stop=True)
    d2s = gs.tile([H, P], dt.float32, tag="d2s")
    nc.scalar.activation(d2s[:], d2[:], AF.Identity, bias=csb["db2c"][:])
    oT = gpB.tile([P, H], dt.float32, tag="pscr2")
    nc.tensor.transpose(oT[:], d2s[:], ident[:H, :H])
    nc.vector.tensor_copy(opk[:, H * wl:H * (wl + 1)], oT[:])


LAST_BENCH_NS = None


def _bench_ns(nc, in_maps, n_cores, iters=16, reps=9):
    """Median wall-time per back-to-back NEFF execution, via a lax.scan that
    threads the output buffers as carry (serializes + defeats CSE)."""
    import time as _time

    import jax
    import jax.numpy as jnp
    from jax.experimental.shard_map import shard_map
    from jax.sharding import Mesh, NamedSharding, PartitionSpec

    import concourse.mybir as mybir
    from concourse import bass2jax

    bass2jax.install_neuronx_cc_hook()
    partition_name = (nc.partition_id_tensor.name
                      if nc.partition_id_tensor else None)
    in_names, out_names, out_avals, zero_outs = [], [], [], []
    for alloc in nc.m.functions[0].allocations:
        if not isinstance(alloc, mybir.MemoryLocationSet):
            continue
        name = alloc.memorylocations[0].name
        if alloc.kind == "ExternalInput":
            if name != partition_name:
                in_names.append(name)
        elif alloc.kind == "ExternalOutput":
            out_names.append(name)
            shape = tuple(alloc.tensor_shape)
            dtype = mybir.dt.np(alloc.dtype)
            out_avals.append(jax.core.ShapedArray(shape, dtype))
            zero_outs.append(np.zeros(shape, dtype))
    n_params = len(in_names)
    n_outs = len(out_names)
    all_names = tuple(in_names + out_names +
                      ([partition_name] if partition_name else []))

    def _fn(*args):
        ins = args[:n_params]
        zouts = tuple(args[n_params:])
        operands = list(ins) + list(zouts)
        if partition_name:
            operands.append(bass2jax.partition_id_tensor())
        outs = bass2jax._bass_exec_p.bind(
            *operands, out_avals=tuple(out_avals), in_names=all_names,
            out_names=tuple(out_names), lowering_input_output_aliases=(),
            sim_require_finite=True, sim_require_nnan=True, nc=nc)
        return tuple(outs)

    devices = jax.devices()[:n_cores]
    mesh = Mesh(np.asarray(devices), ("core",))
    spec = PartitionSpec("core")
    sh = NamedSharding(mesh, spec)
    in_specs = (spec,) * (n_params + n_outs)
    out_specs = (spec,) * n_outs
    donate = tuple(range(n_params, n_params + n_outs))
    concat_in = [
        jax.device_put(np.concatenate(
            [np.asarray(in_maps[c][nm]) for c in range(n_cores)], axis=0), sh)
        for nm in in_names]

    def timed():
        fn = jax.jit(shard_map(_fn, mesh=mesh,
                               in_specs=in_specs, out_specs=out_specs,
                               check_rep=False),
                     donate_argnums=donate, keep_unused=True)
        zss = [[jax.device_put(
            np.zeros((n_cores * z.shape[0], *z.shape[1:]), z.dtype), sh)
            for z in zero_outs] for _ in range(reps + 2)]
        ts = []
        for zs in zss:
            t0 = _time.perf_counter()
            out = fn(*concat_in, *zs)
            jax.block_until_ready(out)
            ts.append(_time.perf_counter() - t0)
        ts = sorted(ts[2:])
        return ts[len(ts) // 2], min(ts)

    med, mn = timed()
    return med * 1e9, mn * 1e9


def kernel(**inputs):
    global LAST_RESULTS
    from concourse.bass_utils import run_bass_kernel_spmd
    from concourse.bass_interp import get_hw_module

    geom, cores, consts = _prep_host(inputs)
    nc = _build(geom)
    nc.m = get_hw_module(nc.m)

    in_maps = []
    for core in cores:
        m = dict(core)
        m.update(consts)
        in_maps.append(m)

    res = run_bass_kernel_spmd(nc, in_maps, core_ids=list(range(geom["NC"])))
    LAST_RESULTS = res

    if os.environ.get("GAT_BENCH"):
        global LAST_BENCH_NS
        LAST_BENCH_NS = _bench_ns(nc, in_maps, geom["NC"])

    N, shard, shard_pad = geom["N"], geom["shard"], geom["shard_pad"]
    out = np.empty((N, H), np.float32)
    for r in range(geom["NC"]):
        out[r * shard:(r + 1) * shard] = res.results[r]["out"][:shard]
    return out

